# revision 66
# baseline (speedup 1.0000x reference)
"""Trainium2 Bass kernel for causal self-attention with doubled rotary.

Full-input contract: kernel(**inputs) takes the complete tensors
(x [4,2048,2048], wq/wk/wv/wo [2048,2048]) and returns [4,2048,2048] fp32.

Sharding: 8 cores = 4 batch elements x 2 head-halves (8 heads each).
Each core computes a partial output projection (its heads' columns of wo);
the host sums the two partials per batch element.

Per-core structure (engine streams execute in emission order, so independent
work is interleaved at emission time to keep the PE dense):
  - group g in 0..3 owns heads (2g, 2g+1): projections q/k/v (contraction
    over embd, bf16 matmuls), doubled-angle rotary on DVE (the reference
    applies rotary twice; R(t)^2 == R(2t)), all into double-buffered SBUF
    tiles. Group 0's weight/x DMAs are split and interleaved so the first
    matmuls start after ~1.5us instead of waiting for the full 5MB.
  - attention pair g-1 is emitted interleaved with the projection of group
    g: QK^T computed transposed (ST[s,t]) so exp(ST) feeds the PV matmul
    directly with v as the stationary operand, software-pipelined 2 deep
    (exp of chunk i overlaps PV of chunk i-2). Causal diagonal chunks are
    sliced at the 128-column grid; the residual triangle is a 0/1 fp16
    multiply on the exp tile (DVE 2x). exp chunks (fp16) accumulate into a
    per-panel fp16 SBUF tile on DVE; a gpsimd partition_all_reduce does the
    reduce+broadcast of the softmax denominator, deferred past the next
    panel's first chunks. Normalized y panels (bf16) stay resident in
    SBUF -- no DRAM spill. Dummy matmuls during the startup DMA wait warm
    the PE clock (HAM ramp).
  - the last pair is interleaved with the first half of the output
    projection; the output projection reads the resident y tiles directly
    and streams the final tile out in narrowing slivers.

Projections run in bf16 (same PE rate as fp32r, half the DMA/SBUF);
attention moving operands are bf16/fp16 (full rate at any width). PSUM
accumulation is fp32 throughout. Output partials are fp32.
"""

import os
import sys

for _p in ("/opt/trn_rl_repo", "/root/.axon_site/_ro/trn_rl_repo"):
    if os.path.isdir(_p) and _p not in sys.path:
        sys.path.insert(0, _p)

import numpy as np

import concourse.bass as bass
import concourse.bass_isa as bass_isa
import concourse.mybir as mybir
from concourse import bacc
from concourse.bass import ds
from concourse.tile import TileContext
from concourse.bass_utils import run_bass_kernel_spmd

F32 = mybir.dt.float32
F32R = mybir.dt.float32r
BF16 = mybir.dt.bfloat16
FP16 = mybir.dt.float16

P = 128          # partitions / head dim
T = 2048         # sequence length
E = 2048         # embedding dim
B = 4
HPC = 8          # heads per core
D = 128          # head dim
PAN = 512        # attention t-panel width (PSUM bank limit for fp32)
NPAN = T // PAN  # 4
XPAN = 512       # projection t-panel width
NXP = T // XPAN  # 4
EO = E // P      # 16 contraction chunks for projections
NGRP = 4         # head pairs per core
NCH = T // P     # 16 s-chunks (also v t-tiles)
SCALE = 1.0 / float(np.sqrt(D))
NEG = -1.0e9

ADD = mybir.AluOpType.add
MULT = mybir.AluOpType.mult
DIV = mybir.AluOpType.divide
EXP = mybir.ActivationFunctionType.Exp


def _zip_emit(*lists):
    """Emit thunks from several lists round-robin, proportionally."""
    lists = [list(l) for l in lists if l]
    if not lists:
        return
    total = max(len(l) for l in lists)
    idx = [0.0] * len(lists)
    step = [len(l) / total for l in lists]
    for _ in range(total):
        for li, l in enumerate(lists):
            idx[li] += step[li]
            while idx[li] >= 1.0 and l:
                l.pop(0)()
                idx[li] -= 1.0
    for l in lists:
        for f in l:
            f()


class Ctx:
    pass


def build_program():
    nc = bacc.Bacc()
    cx = Ctx()
    cx.nc = nc

    cx.xT = nc.declare_dram_parameter("xT", [E, T], BF16, isOutput=False)
    cx.wqT = nc.declare_dram_parameter("wqT", [E, HPC * D], BF16, isOutput=False)
    cx.wkT = nc.declare_dram_parameter("wkT", [E, HPC * D], BF16, isOutput=False)
    cx.wvT = nc.declare_dram_parameter("wvT", [E, HPC * D], BF16, isOutput=False)
    cx.woT = nc.declare_dram_parameter("woT", [HPC * D, E], BF16, isOutput=False)
    cx.cos2 = nc.declare_dram_parameter("cos2", [P, T], FP16, isOutput=False)
    cx.sin2 = nc.declare_dram_parameter("sin2", [P, T], FP16, isOutput=False)
    cx.mask = nc.declare_dram_parameter("mask", [P, P], FP16, isOutput=False)
    cx.out = nc.declare_dram_parameter("out", [E, T], F32, isOutput=True)

    with TileContext(nc) as tc:
        cx.tc = tc
        with tc.tile_pool(name="const", bufs=1) as cpool:
            cx.mk = cpool.tile([P, P], FP16, tag="mk")
            om_f = cpool.tile([P, P], F32, tag="om_f")
            nc.vector.memset(om_f, 1.0)
            # preload the exp table set while the PE is still waiting on
            # the first weight DMAs
            dum = cpool.tile([P, 16], FP16, tag="dum")
            nc.scalar.activation(dum, om_f[:, 0:16], EXP)
            # warm the PE clock (HAM/p-state ramp) during the startup DMA
            # wait with dummy matmuls; the result is never read
            with tc.tile_pool(name="psW", bufs=1, space="PSUM") as psW:
                wt = psW.tile([P, P], F32, tag="wt")
                for i in range(10):
                    nc.tensor.matmul(
                        wt, lhsT=om_f, rhs=om_f, start=(i == 0),
                        stop=(i == 9), skip_group_check=True,
                    )

            with (
                tc.tile_pool(name="ex", bufs=8) as expool,
                tc.tile_pool(name="da", bufs=6) as dapool,
                tc.tile_pool(name="dn1", bufs=2) as dn1pool,
                tc.tile_pool(name="ysb", bufs=1) as ypool,
                tc.tile_pool(name="psS", bufs=3, space="PSUM") as psS,
                tc.tile_pool(name="psY", bufs=3, space="PSUM") as psY,
                tc.tile_pool(name="qk", bufs=2) as qkpool,
                tc.tile_pool(name="vp", bufs=2) as vpool,
            ):
                cx.expool, cx.dapool, cx.dn1pool = expool, dapool, dn1pool
                cx.ypool = ypool
                cx.psS, cx.psY = psS, psY
                cx.qkpool, cx.vpool = qkpool, vpool
                cx.qkv = {}  # g -> (qT, kT, v_sb)
                cx.ysb = {}  # (h, jp) -> resident SBUF y tile

                with (
                    tc.tile_pool(name="tab", bufs=1) as tabpool,
                    tc.tile_pool(name="xp", bufs=2) as xpool,
                    tc.tile_pool(name="wp", bufs=2) as wpool,
                    tc.tile_pool(name="rot", bufs=2) as rotpool,
                    tc.tile_pool(name="sw", bufs=1) as swpool,
                    tc.tile_pool(name="psP", bufs=2, space="PSUM") as psP,
                ):
                    cx.xpool, cx.wpool = xpool, wpool
                    cx.rotpool, cx.swpool, cx.psP = rotpool, swpool, psP

                    def load_tables():
                        cx.c2 = tabpool.tile([P, T], FP16, tag="c2")
                        nc.scalar.dma_start(cx.c2, cx.cos2[:, :])
                        cx.s2 = tabpool.tile([P, T], FP16, tag="s2")
                        nc.scalar.dma_start(cx.s2, cx.sin2[:, :])
                        nc.scalar.dma_start(cx.mk, cx.mask[:, :])

                    p0 = _proj_thunks(cx, 0)
                    # rotary tables aren't needed until the first rot_panel
                    # (~12us in); emit their DMAs after the startup-critical
                    # weight/x chunks so they don't head-of-line block HWDGE
                    p0.insert(4, load_tables)
                    for f in p0:
                        f()
                    for g in range(1, NGRP):
                        _zip_emit(_proj_thunks(cx, g), _attn_thunks(cx, g - 1))

                with (
                    tc.tile_pool(name="wo", bufs=2) as wopool,
                    tc.tile_pool(name="ob", bufs=3) as opool,
                    tc.tile_pool(name="psO", bufs=2, space="PSUM") as psO,
                ):
                    cx.wopool, cx.opool, cx.psO = wopool, opool, psO
                    cx.wo_half = {}
                    _load_wo_half(cx, 0)
                    # outproj(0, jp) may only be emitted after pair-3 has
                    # finalized panel jp (it reads the resident y tiles for
                    # heads 6/7): interleave panel jp's outproj with panel
                    # jp+1's chunks.
                    panels = [_attn_thunks(cx, NGRP - 1, only_jp=jp)
                              for jp in range(NPAN)]
                    oproj0 = [_outproj_thunks(cx, 0, only_jp=jp)
                              for jp in range(NPAN)]
                    for f in panels[0]:
                        f()
                    for jp in range(1, NPAN):
                        _zip_emit(panels[jp], oproj0[jp - 1])
                    _load_wo_half(cx, 1)
                    for f in oproj0[NPAN - 1]:
                        f()
                    for f in _outproj_thunks(cx, 1):
                        f()

    nc.finalize()
    return nc


def _proj_thunks(cx, g):
    """Thunk list for group g's projections + rotary."""
    nc = cx.nc
    thunks = []
    state = {}

    wq_r = cx.wqT.rearrange("(eo p) d -> p eo d", p=P)
    wk_r = cx.wkT.rearrange("(eo p) d -> p eo d", p=P)
    wv_r = cx.wvT.rearrange("(eo p) d -> p eo d", p=P)
    x_r = cx.xT.rearrange("(eo p) t -> p eo t", p=P)
    dsl = ds(g * 2 * D, 2 * D)

    def alloc_group():
        wq_sb = cx.wpool.tile([P, EO, 2 * D], BF16, tag="wq")
        wk_sb = cx.wpool.tile([P, EO, 2 * D], BF16, tag="wk")
        wv_sb = cx.wpool.tile([P, EO, 2 * D], BF16, tag="wv")
        qT = cx.qkpool.tile([P, 2, T], BF16, tag="qT")
        kT = cx.qkpool.tile([P, 2, T], BF16, tag="kT")
        v_sb = cx.vpool.tile([P, NCH, 2 * D], FP16, tag="v")
        cx.qkv[g] = (qT, kT, v_sb)
        cx._w = (wq_sb, wk_sb, wv_sb)

    def load_panel(xj):
        def f():
            xp = cx.xpool.tile([P, EO, XPAN], BF16, tag="xp")
            nc.sync.dma_start(xp, x_r[:, :, ds(xj * XPAN, XPAN)])
            state[xj] = xp
        return f

    if g == 0:
        # split + interleave the first weight/x DMAs so the first matmuls
        # can start after the first chunks land (weights on the sync queue,
        # x panels on the vector queue)
        def start0():
            alloc_group()
            xp = cx.xpool.tile([P, EO, XPAN], BF16, tag="xp")
            state[0] = xp
            wq_sb = cx._w[0]
            for c in range(4):
                eos = ds(c * 4, 4)
                nc.sync.dma_start(wq_sb[:, eos, :], wq_r[:, eos, dsl])
                for h in range(2):
                    eoh = ds(c * 4 + h * 2, 2)
                    nc.sync.dma_start(
                        xp[:, eoh, :], x_r[:, eoh, ds(0, XPAN)]
                    )

        def wload(wi):
            def f():
                src = (wk_r, wv_r)[wi - 1]
                for c in range(2):
                    eos = ds(c * 8, 8)
                    nc.sync.dma_start(cx._w[wi][:, eos, :], src[:, eos, dsl])
            return f

        thunks.append(start0)
    else:
        def startg():
            alloc_group()
            nc.sync.dma_start(cx._w[0], wq_r[:, :, dsl])
            nc.sync.dma_start(cx._w[1], wk_r[:, :, dsl])
            nc.sync.dma_start(cx._w[2], wv_r[:, :, dsl])

        thunks.append(startg)

    def qk_group(xj, wi, hl):
        def f():
            xp = state[xj]
            w_sb = cx._w[wi]
            dst = cx.qkv[g][wi]
            ps = cx.psP.tile([P, PAN], F32, tag="psP")
            psq = ps[:, :XPAN]
            for eo in range(EO):
                nc.tensor.matmul(
                    psq,
                    lhsT=w_sb[:, eo, ds(hl * D, D)],
                    rhs=xp[:, eo, :],
                    start=(eo == 0),
                    stop=(eo == EO - 1),
                )
            nc.vector.tensor_copy(dst[:, hl, ds(xj * XPAN, XPAN)], psq)
        return f

    def v_group(xj, tt):
        def f():
            xp = state[xj]
            wv_sb = cx._w[2]
            v_sb = cx.qkv[g][2]
            ps = cx.psP.tile([P, PAN], F32, tag="psP")
            psv = ps[:, : 2 * D]
            for eo in range(EO):
                nc.tensor.matmul(
                    psv,
                    lhsT=xp[:, eo, ds(tt * P, P)],
                    rhs=wv_sb[:, eo, :],
                    start=(eo == 0),
                    stop=(eo == EO - 1),
                )
            nc.scalar.copy(v_sb[:, xj * (XPAN // P) + tt, :], psv)
        return f

    def rot_panel(src_i, hl, jp):
        def f():
            src = cx.qkv[g][src_i]
            sl = ds(jp * PAN, PAN)
            qsw = cx.swpool.tile([P, PAN], BF16, tag="qsw")
            nc.sync.dma_start(qsw[0:64, :], src[64:128, hl, sl])
            nc.sync.dma_start(qsw[64:128, :], src[0:64, hl, sl])
            tmp = cx.rotpool.tile([P, PAN], BF16, tag="rtmp")
            nc.vector.tensor_tensor(tmp, qsw[:, :], cx.s2[:, sl], op=MULT)
            nc.vector.tensor_tensor(
                src[:, hl, sl], src[:, hl, sl], cx.c2[:, sl], op=MULT
            )
            nc.vector.tensor_tensor(src[:, hl, sl], src[:, hl, sl], tmp, op=ADD)
        return f

    for xj in range(NXP):
        if not (g == 0 and xj == 0):
            thunks.append(load_panel(xj))
        for hl in range(2):
            thunks.append(qk_group(xj, 0, hl))
        if g == 0 and xj == 0:
            thunks.append(wload(1))
        for hl in range(2):
            thunks.append(qk_group(xj, 1, hl))
        if g == 0 and xj == 0:
            thunks.append(wload(2))
        for tt in range(XPAN // P):
            thunks.append(v_group(xj, tt))
        for src_i in range(2):
            for hl in range(2):
                thunks.append(rot_panel(src_i, hl, xj))
    return thunks


def _attn_thunks(cx, g, only_jp=None):
    """Thunk list for the attention of head pair g (heads 2g, 2g+1)."""
    nc = cx.nc
    thunks = []
    st8 = cx.__dict__.setdefault(f"_attn_state_{g}", {})

    exq = cx.__dict__.setdefault(f"_attn_ex_{g}", {})

    def chunk_a(hl, jp, i):
        """QK^T -> mask -> exp for chunk i (one step ahead of chunk_b)."""
        def f():
            qT, kT, v_sb = cx.qkv[g]
            if i == 0:
                ytp = cx.psY.tile([P, PAN], F32, tag="psY")
                dacc = cx.dapool.tile([P, PAN], FP16, tag="da")
                st8[(hl, jp)] = (ytp, dacc)
            di = i - 4 * jp
            off = P * di if di > 0 else 0
            w = PAN - off
            st = cx.psS.tile([P, PAN], F32, tag="psS")
            stw = st[:, off:PAN]
            nc.tensor.matmul(
                stw,
                lhsT=kT[:, hl, ds(i * P, P)],
                rhs=qT[:, hl, ds(jp * PAN + off, w)],
                start=True,
                stop=True,
            )
            ex = cx.expool.tile([P, PAN], FP16, tag="ex")
            exw = ex[:, off:PAN]
            nc.scalar.activation(exw, stw, EXP, scale=SCALE)
            if di >= 0:
                # causal mask as a 0/1 multiply on the fp16 exp tile (DVE
                # 2x) -- keeps the mask off the PE entirely
                nc.vector.tensor_tensor(
                    ex[:, off:off + P], ex[:, off:off + P], cx.mk, op=MULT
                )
            exq[(hl, i)] = ex
        return f

    def chunk_b(hl, jp, i):
        """PV accumulate + denominator accumulate for chunk i."""
        def f():
            qT, kT, v_sb = cx.qkv[g]
            nch = 4 * jp + 4
            ytp, dacc = st8[(hl, jp)]
            di = i - 4 * jp
            off = P * di if di > 0 else 0
            ex = exq.pop((hl, i))
            exw = ex[:, off:PAN]
            nc.tensor.matmul(
                ytp[:, off:PAN],
                lhsT=v_sb[:, i, ds(hl * D, D)],
                rhs=exw,
                start=(i == 0),
                stop=(i == nch - 1),
            )
            # softmax denominator: accumulate exp chunks on DVE (fp16 2x),
            # partition-reduced once per panel in finalize
            if i == 0:
                nc.vector.tensor_copy(dacc, ex)
            else:
                nc.vector.tensor_tensor(
                    dacc[:, off:PAN], dacc[:, off:PAN], exw, op=ADD
                )
        return f

    def finalize(hl, jp):
        def f():
            h = 2 * g + hl
            ytp, dacc = st8.pop((hl, jp))
            # partition-sum + broadcast of the softmax denominator on the
            # otherwise-idle gpsimd engine (was an all-ones matmul on the PE)
            dps = cx.dn1pool.tile([P, PAN], F32, tag="dps")
            nc.gpsimd.partition_all_reduce(
                dps[:, :], dacc[:, :], P, bass_isa.ReduceOp.add
            )
            rdb = cx.dn1pool.tile([P, PAN], F32, tag="rdb")
            nc.vector.reciprocal_approx_fast(out=rdb, in_=dps)
            yts = cx.ypool.tile([P, PAN], BF16, tag=f"y{h}_{jp}")
            nc.vector.tensor_tensor(yts, ytp, rdb, op=MULT)
            cx.ysb[(h, jp)] = yts
        return f

    jps = range(NPAN) if only_jp is None else [only_jp]
    pending_fin = []
    for jp in jps:
        nch = 4 * jp + 4
        # 2-step software pipeline: PV of chunk i-2 runs on the PE while
        # the ACT engine computes exp of chunks i-1/i
        panel = []
        for i in range(nch + 2):
            for hl in range(2):
                if i < nch:
                    panel.append(chunk_a(hl, jp, i))
                if i > 1:
                    panel.append(chunk_b(hl, jp, i - 2))
        # defer the previous panel's finalize past this panel's first QK
        # pair so the denominator reduce doesn't wait on the DVE drain
        thunks.extend(panel[:2])
        thunks.extend(pending_fin)
        thunks.extend(panel[2:])
        pending_fin = [finalize(0, jp), finalize(1, jp)]
    thunks.extend(pending_fin)
    return thunks


def _load_wo_half(cx, half):
    nc = cx.nc
    wo_sb = cx.wopool.tile([P, HPC, E // 2], BF16, tag="wo")
    nc.sync.dma_start(
        wo_sb,
        cx.woT.rearrange("(c p) e -> p c e", p=P)[:, :, ds(half * (E // 2), E // 2)],
    )
    cx.wo_half[half] = wo_sb


def _outproj_thunks(cx, half, only_jp=None):
    """Thunk list for the output projection over e-tiles of one wo half."""
    nc = cx.nc
    thunks = []

    def etile(jp, et, split=False):
        def f():
            wo_sb = cx.wo_half[half]
            # the very last etile streams out in narrowing slivers so the
            # final store's flush is short and overlaps earlier matmuls
            cols = (ds(0, 256), ds(256, 128), ds(384, 128)) if split \
                else (ds(0, PAN),)
            for cs in cols:
                ps = cx.psO.tile([P, cs.size], F32, tag="psO")
                for dc in range(HPC):
                    nc.tensor.matmul(
                        ps,
                        lhsT=wo_sb[:, dc, ds((et - half * 8) * P, P)],
                        rhs=cx.ysb[(dc, jp)][:, cs],
                        start=(dc == 0),
                        stop=(dc == HPC - 1),
                    )
                ob = cx.opool.tile([P, cs.size], F32,
                                   tag="obh" if split else "ob")
                nc.scalar.copy(ob, ps)
                nc.sync.dma_start(
                    cx.out[ds(et * P, P),
                           ds(jp * PAN + cs.start, cs.size)], ob
                )
        return f

    jps = range(NPAN) if only_jp is None else [only_jp]
    for jp in jps:
        for et in range(half * 8, half * 8 + 8):
            last = half == 1 and jp == NPAN - 1 and et == half * 8 + 7
            thunks.append(etile(jp, et, split=last))
    return thunks


def make_tables():
    j = np.arange(0, D, 2, dtype=np.float64) / D
    inv_freq = 1.0 / (10000.0 ** j)
    t = np.arange(T, dtype=np.float64)
    fr = np.outer(t, inv_freq)                            # [T, 64]
    c2 = np.cos(2.0 * fr).T                               # [64, T]
    s2 = np.sin(2.0 * fr).T
    cos2 = np.concatenate([c2, c2], axis=0).astype(np.float16)
    sin2 = np.concatenate([s2, -s2], axis=0).astype(np.float16)
    return cos2, sin2


def make_mask():
    s = np.arange(P)[:, None]
    c = np.arange(P)[None, :]
    return np.where(s <= c, 1.0, 0.0).astype(np.float16)


def make_in_maps(x, wq, wk, wv, wo):
    import ml_dtypes
    BF = ml_dtypes.bfloat16
    cos2, sin2 = make_tables()
    mask = make_mask()
    xb = [np.ascontiguousarray(x[b].T).astype(BF) for b in range(B)]
    in_maps = []
    for c in range(8):
        b, hh = c // 2, c % 2
        rows = slice(hh * HPC * D, (hh + 1) * HPC * D)
        in_maps.append({
            "xT": xb[b],
            "wqT": np.ascontiguousarray(wq[rows].T).astype(BF),
            "wkT": np.ascontiguousarray(wk[rows].T).astype(BF),
            "wvT": np.ascontiguousarray(wv[rows].T).astype(BF),
            "woT": np.ascontiguousarray(wo[:, rows].T).astype(BF),
            "cos2": cos2,
            "sin2": sin2,
            "mask": mask,
        })
    return in_maps


_PROGRAM_CACHE = {}


def get_program():
    if "nc" not in _PROGRAM_CACHE:
        _PROGRAM_CACHE["nc"] = build_program()
    return _PROGRAM_CACHE["nc"]


def kernel(x, wq, wk, wv, wo, _results_hook=None):
    x = np.asarray(x, dtype=np.float32)
    wq = np.asarray(wq, dtype=np.float32)
    wk = np.asarray(wk, dtype=np.float32)
    wv = np.asarray(wv, dtype=np.float32)
    wo = np.asarray(wo, dtype=np.float32)

    nc = get_program()
    in_maps = make_in_maps(x, wq, wk, wv, wo)
    res = run_bass_kernel_spmd(nc, in_maps, list(range(8)))
    if _results_hook is not None:
        _results_hook(res)
    outs = [r["out"] for r in res.results]
    full = np.empty((B, T, E), dtype=np.float32)
    for b in range(B):
        full[b] = (outs[2 * b] + outs[2 * b + 1]).T
    return full
